# revision 21
# baseline (speedup 1.0000x reference)
"""Trainium2 Bass kernel for ExplainableDumplingGNN (MPNN -> 3x GAT -> SAGE -> pool).

Self-contained: takes full inputs, shards node blocks + incident edges across
8 NeuronCores internally, runs one SPMD Bass kernel, returns [64, 2] log-probs.

Sharding: core c owns nodes [1250c, 1250c+1250), padded to 1280 rows (10 dst
blocks of 128). Edges are assigned to the core owning their dst, sorted by
dst, grouped per 128-node dst block, padded to K 128-edge tiles per block.

Per-edge src features (xl[src]) are fetched with the gpsimd dma_gather ucode
(the serial GPSIMD descriptor-gen cost is the kernel's main constraint, so
only ONE gather per GAT layer). The dst-side xr[dst] per edge is produced on
the tensor engine: z = selT^T @ xr_block + I @ xg accumulated in PSUM, where
selT is a host-built one-hot [dst, edge]. leaky_relu(0.2) is a single
scalar-engine Prelu reading PSUM. Attention softmax uses exp(alpha) with no
max subtraction (alpha stays bounded for this input family). The weighted
scatter-add per dst block is a one-hot matmul (sel = PE-transposed selT)
accumulating in PSUM. Features/matmuls bf16 with fp32 accumulation.
"""
import sys

sys.path.insert(0, "/opt/trn_rl_repo")

import ml_dtypes
import numpy as np

import concourse.bacc as bacc
import concourse.bass as bass
import concourse.mybir as mybir
import concourse.tile as tile
from concourse import bass_utils
from concourse.masks import make_identity

P = 128
NCORES = 8
N = 10000
NBLK = 1250
NPAD = 1280
BLOCKS = 10
NFULL = NPAD * NCORES  # 10240
D_IN = 8
HID = 64
HEADS = 8
HC = 512
G = 64
XG = 128  # padded x row for MPNN dma_gather (256B bf16 rows)
MAXT = 8  # max 128-edge tiles per dma_gather (1024 descriptors)

F32 = mybir.dt.float32
BF = mybir.dt.bfloat16
I16 = mybir.dt.int16

BF_NP = ml_dtypes.bfloat16

_CACHE = {}

AF = mybir.ActivationFunctionType


def _chunks(K):
    out = []
    k0 = 0
    while k0 < K:
        n = min(MAXT, K - k0)
        out.append((k0, n))
        k0 += n
    return out


def _groupsn(K, n):
    out = []
    k0 = 0
    while k0 < K:
        m = min(n, K - k0)
        out.append((k0, m))
        k0 += m
    return out


def _pad_id(n):
    return (n // NBLK) * NPAD + (n % NBLK)


def _split_blocks(es_pad, ed_local):
    order = np.argsort(ed_local, kind="stable")
    es_pad, ed_local = es_pad[order], ed_local[order]
    per_block = []
    K = 1
    for b in range(BLOCKS):
        m = (ed_local >= b * P) & (ed_local < (b + 1) * P)
        s, d = es_pad[m], ed_local[m] - b * P
        # within a block, edge order is free: sort by src so each gather's
        # descriptors hit HBM in ascending address order (row-buffer hits)
        o = np.argsort(s, kind="stable")
        s, d = s[o], d[o]
        per_block.append((s, d))
        K = max(K, (len(s) + P - 1) // P)
    return per_block, K


def _pack_idx16(flat):
    """[n] int -> [128, n//16] int16, wrapped in 16 partitions, replicated x8."""
    n = len(flat)
    ncols = n // 16
    a = np.zeros((P, ncols), np.int16)
    j = np.arange(n)
    a[j % 16, j // 16] = flat.astype(np.int16)
    for c in range(1, 8):
        a[16 * c:16 * (c + 1)] = a[:16]
    return a


def _finalize_edge_arrays(per_block, K):
    """Returns (src_flat, selT [d,e], sel [e,d], mask [P, T])."""
    T = BLOCKS * K
    src_flat = np.zeros((BLOCKS, K * P), np.int32)
    selT = np.zeros((P, T * P), BF_NP)
    sel = np.zeros((P, T * P), BF_NP)
    mask = np.zeros((P, T), BF_NP)
    for b, (s, d) in enumerate(per_block):
        n = len(s)
        slots = K * P
        s_pad = np.zeros(slots, np.int32)
        d_pad = np.zeros(slots, np.int32)
        m_pad = np.zeros(slots, np.float32)
        s_pad[:n] = s
        d_pad[:n] = d
        m_pad[:n] = 1.0
        if 0 < n < slots:
            s_pad[n:] = s[n - 1]
            d_pad[n:] = d[n - 1]
        src_flat[b] = s_pad
        for k in range(K):
            t = b * K + k
            sl = slice(k * P, (k + 1) * P)
            dk = d_pad[sl]
            selT[dk, t * P + np.arange(P)] = BF_NP(1.0)
            sel[np.arange(P), t * P + dk] = BF_NP(1.0)
            mask[:, t] = m_pad[sl].astype(BF_NP)
    return src_flat, selT, sel, mask


def _finalize_sel_masked(per_block, K):
    """sel [128 e, BLOCKS*K*128 d] bf16 with the pad-edge mask baked in."""
    sel = np.zeros((P, BLOCKS * K * P), BF_NP)
    src_flat = np.zeros((BLOCKS, K * P), np.int32)
    for b, (s, d) in enumerate(per_block):
        n = len(s)
        slots = K * P
        s_pad = np.zeros(slots, np.int32)
        d_pad = np.zeros(slots, np.int32)
        s_pad[:n] = s
        d_pad[:n] = d
        if 0 < n < slots:
            s_pad[n:] = s[n - 1]
            d_pad[n:] = d[n - 1]
        src_flat[b] = s_pad
        for k in range(K):
            t = b * K + k
            sl = slice(k * P, (k + 1) * P)
            dk = d_pad[sl]
            valid = (np.arange(k * P, (k + 1) * P) < n)
            e = np.arange(P)[valid]
            sel[e, t * P + dk[valid]] = BF_NP(1.0)
    return src_flat, sel


def _pack_block_idx(src_flat, K):
    """src_flat [BLOCKS, K*P] -> packed int16 [128, BLOCKS * K*P//16]."""
    cols = K * P // 16
    out = np.zeros((P, BLOCKS * cols), np.int16)
    for b in range(BLOCKS):
        out[:, b * cols:(b + 1) * cols] = _pack_idx16(src_flat[b])
    return out


def _preprocess(inputs):
    x = np.asarray(inputs["x"], np.float32)
    ei = np.asarray(inputs["edge_index"], np.int32)
    batch = np.asarray(inputs["batch"], np.int32)
    src, dst = ei[0], ei[1]

    blocks_per_core = []
    K_gat = 1
    K_sage = 1
    for c in range(NCORES):
        lo, hi = c * NBLK, (c + 1) * NBLK
        m = (dst >= lo) & (dst < hi)
        s_c = _pad_id(src[m]).astype(np.int32)
        d_c = (dst[m] - lo).astype(np.int32)
        own = np.arange(lo, hi, dtype=np.int32)
        gs = np.concatenate([s_c, _pad_id(own).astype(np.int32)])
        gd = np.concatenate([d_c, (own - lo)])
        gat_blocks, kg = _split_blocks(gs, gd)
        sage_blocks, ks = _split_blocks(s_c, d_c)
        K_gat = max(K_gat, kg)
        K_sage = max(K_sage, ks)
        blocks_per_core.append((gat_blocks, sage_blocks, d_c))

    per_core = []
    for c in range(NCORES):
        gat_blocks, sage_blocks, d_c = blocks_per_core[c]
        gsrc_flat, gselT, gsel, gmask = _finalize_edge_arrays(gat_blocks, K_gat)
        ssrc_flat, ssel = _finalize_sel_masked(sage_blocks, K_sage)
        cnt = np.bincount(d_c, minlength=NPAD).astype(np.float32)
        rc = 1.0 / np.maximum(cnt, 1.0)
        recip_cnt = rc.reshape(BLOCKS, P).T.copy()  # [p, b]
        per_core.append(dict(
            gat_idx16=_pack_block_idx(gsrc_flat, K_gat),
            sage_idx16=_pack_block_idx(ssrc_flat, K_sage),
            gat_selT=gselT, gat_sel=gsel, gat_mask=gmask, sage_sel=ssel,
            recip_cnt=recip_cnt.astype(np.float32),
        ))

    B_all = []
    for c in range(NCORES):
        Bm = np.zeros((P, BLOCKS * G), np.float32)
        loc = np.arange(NBLK)
        gids = batch[c * NBLK:(c + 1) * NBLK]
        Bm[loc % P, (loc // P) * G + gids] = 1.0
        B_all.append(Bm.astype(BF_NP))

    gcnt = np.bincount(batch, minlength=G).astype(np.float32)
    recip_gcnt = (1.0 / np.maximum(gcnt, 1.0)).reshape(G, 1).astype(np.float32)

    x_gather = np.zeros((NFULL, XG), BF_NP)
    for c in range(NCORES):
        x_gather[c * NPAD:c * NPAD + NBLK, :D_IN] = x[c * NBLK:(c + 1) * NBLK]
    x_gather[:, D_IN] = BF_NP(1.0)
    xT_aug = []
    for c in range(NCORES):
        t = np.zeros((D_IN + 1, NPAD), np.float32)
        t[:D_IN, :NBLK] = x[c * NBLK:(c + 1) * NBLK].T
        t[D_IN, :] = 1.0
        xT_aug.append(t)

    w = {}
    w["mlw_aug"] = np.concatenate(
        [np.asarray(inputs["mpnn_lin_w"], np.float32),
         np.asarray(inputs["mpnn_lin_b"], np.float32)[None, :]], axis=0)
    w["muw"] = np.asarray(inputs["mpnn_upd_w"], np.float32)
    w["mub_col"] = np.asarray(inputs["mpnn_upd_b"], np.float32).reshape(HID, 1)
    for i in (1, 2, 3):
        w[f"wl{i}"] = np.asarray(inputs[f"g{i}_wl"], np.float32).astype(BF_NP)
        w[f"wr{i}"] = np.asarray(inputs[f"g{i}_wr"], np.float32).astype(BF_NP)
        w[f"wres{i}"] = np.asarray(inputs[f"g{i}_res"], np.float32).astype(BF_NP)
        w[f"att_rep{i}"] = np.tile(
            np.asarray(inputs[f"g{i}_att"], np.float32).reshape(1, HC),
            (P, 1)).astype(BF_NP)
        w[f"b_rep{i}"] = np.tile(
            np.asarray(inputs[f"g{i}_b"], np.float32)[None, :],
            (P, 1)).astype(BF_NP)
    w["sage_wn"] = np.asarray(inputs["sage_wn"], np.float32).astype(BF_NP)
    w["sage_wr"] = np.asarray(inputs["sage_wr"], np.float32).astype(BF_NP)
    w["sbn_rep"] = np.tile(np.asarray(inputs["sage_bn"], np.float32)[None, :], (P, 1))
    w["out_w"] = np.asarray(inputs["out_w"], np.float32)
    w["ob_rep"] = np.tile(np.asarray(inputs["out_b"], np.float32)[None, :], (G, 1))

    return dict(
        K_gat=K_gat, K_sage=K_sage, per_core=per_core,
        B_all=B_all, recip_gcnt=recip_gcnt,
        x_gather=x_gather, xT_aug=xT_aug, weights=w,
    )


def _build(K_gat, K_sage):
    nc = bacc.Bacc("TRN2", target_bir_lowering=False, debug=False,
                   num_devices=NCORES)

    TG = BLOCKS * K_gat
    TS = BLOCKS * K_sage
    GCOLS = K_gat * P // 16
    SCOLS = K_sage * P // 16

    x_gather_in = nc.dram_tensor("x_gather", [NFULL, XG], BF, kind="ExternalInput")
    xT_aug = nc.dram_tensor("xT_aug", [D_IN + 1, NPAD], F32, kind="ExternalInput")
    gat_idx_in = nc.dram_tensor("gat_idx16", [P, BLOCKS * GCOLS], I16,
                                kind="ExternalInput")
    sage_idx_in = nc.dram_tensor("sage_idx16", [P, BLOCKS * SCOLS], I16,
                                 kind="ExternalInput")
    gat_selT_in = nc.dram_tensor("gat_selT", [P, TG * P], BF, kind="ExternalInput")
    gat_sel_in = nc.dram_tensor("gat_sel", [P, TG * P], BF, kind="ExternalInput")
    gat_mask_in = nc.dram_tensor("gat_mask", [P, TG], BF, kind="ExternalInput")
    sage_sel_in = nc.dram_tensor("sage_sel", [P, TS * P], BF, kind="ExternalInput")
    rcnt_in = nc.dram_tensor("recip_cnt", [P, BLOCKS], F32, kind="ExternalInput")
    B_in = nc.dram_tensor("B_onehot", [P, BLOCKS * G], BF, kind="ExternalInput")
    rgc_in = nc.dram_tensor("recip_gcnt", [G, 1], F32, kind="ExternalInput")

    mlw_aug_in = nc.dram_tensor("mlw_aug", [D_IN + 1, HID], F32, kind="ExternalInput")
    muw_in = nc.dram_tensor("muw", [2 * HID, HID], F32, kind="ExternalInput")
    mub_in = nc.dram_tensor("mub_col", [HID, 1], F32, kind="ExternalInput")
    wls, wrs, wress, atts, brs = {}, {}, {}, {}, {}
    for i in (1, 2, 3):
        ind = HID if i == 1 else HC
        wls[i] = nc.dram_tensor(f"wl{i}", [ind, HC], BF, kind="ExternalInput")
        wrs[i] = nc.dram_tensor(f"wr{i}", [ind, HC], BF, kind="ExternalInput")
        wress[i] = nc.dram_tensor(f"wres{i}", [ind, HC], BF, kind="ExternalInput")
        atts[i] = nc.dram_tensor(f"att_rep{i}", [P, HC], BF, kind="ExternalInput")
        brs[i] = nc.dram_tensor(f"b_rep{i}", [P, HC], BF, kind="ExternalInput")
    swn_in = nc.dram_tensor("sage_wn", [HC, HID], BF, kind="ExternalInput")
    swr_in = nc.dram_tensor("sage_wr", [HC, HID], BF, kind="ExternalInput")
    sbn_in = nc.dram_tensor("sbn_rep", [P, HID], F32, kind="ExternalInput")
    ow_in = nc.dram_tensor("out_w", [HID, 2], F32, kind="ExternalInput")
    ob_in = nc.dram_tensor("ob_rep", [G, 2], F32, kind="ExternalInput")

    out = nc.dram_tensor("out", [G, 2], F32, kind="ExternalOutput")

    gat_ch = _chunks(K_gat)
    sage_ch = _chunks(K_sage)
    gat_g4 = _groupsn(K_gat, 4)
    gat_g2 = _groupsn(K_gat, 2)

    with tile.TileContext(nc) as tc:
        with (
            tc.tile_pool(name="const", bufs=1) as cp,
            tc.tile_pool(name="resid", bufs=1) as rp,
            tc.tile_pool(name="dram", bufs=1, space="DRAM") as dr,
        ):
            ident = cp.tile([P, P], F32)
            make_identity(nc, ident[:])
            ident_bf = cp.tile([P, P], BF)
            nc.vector.tensor_copy(ident_bf[:], ident[:])

            gat_idx = cp.tile([P, BLOCKS * GCOLS], I16)
            nc.sync.dma_start(gat_idx[:], gat_idx_in[:])
            sage_idx = cp.tile([P, BLOCKS * SCOLS], I16)
            nc.sync.dma_start(sage_idx[:], sage_idx_in[:])
            gmask = cp.tile([P, TG], BF)
            nc.sync.dma_start(gmask[:], gat_mask_in[:])
            B_sb = cp.tile([P, BLOCKS * G], BF)
            nc.sync.dma_start(B_sb[:], B_in[:])
            rgc = cp.tile([G, 1], F32)
            nc.sync.dma_start(rgc[:], rgc_in[:])
            rcnt = cp.tile([P, BLOCKS], F32)
            nc.sync.dma_start(rcnt[:], rcnt_in[:])

            # persistent transposed own-node activations
            hT1 = rp.tile([HID, NPAD], BF)       # mpnn out (contract=64)
            hT_a = rp.tile([P, 4 * NPAD], BF)    # gat1 out
            hT_b = rp.tile([P, 4 * NPAD], BF)    # gat2 out / gat3 out (rotate)
            xr_all = rp.tile([P, BLOCKS * HC], BF)
            res_all = rp.tile([P, BLOCKS * HC], BF)

            def gather_block(dst_blk, src_dram, idx_sb, b, chunks, cols, width):
                for (k0, nt) in chunks:
                    nidx = nt * P
                    nc.gpsimd.dma_gather(
                        dst_blk[:, k0 * width:(k0 + nt) * width]
                            .rearrange("p (k d) -> p k d", k=nt),
                        src_dram[:],
                        idx_sb[:, b * cols + k0 * P // 16:
                               b * cols + (k0 + nt) * P // 16],
                        nidx, nidx, width)

            # =========================================================
            # Stage 0: MPNN -> hT1 [64, 1280] bf16 (transposed, resident)
            # h1 = relu(cat(xw, m) @ muw + mub);  relu(lrelu(.,0.1)) == relu
            # =========================================================
            with (
                tc.tile_pool(name="mp_sb", bufs=1) as wp,
                tc.tile_pool(name="mp_ps", bufs=1, space="PSUM") as pp,
            ):
                ssel_sb = wp.tile([P, TS * P], BF)
                nc.sync.dma_start(ssel_sb[:], sage_sel_in[:])
                xT_sb = wp.tile([D_IN + 1, NPAD], F32)
                nc.sync.dma_start(xT_sb[:], xT_aug[:])
                mlw_sb = wp.tile([D_IN + 1, HID], F32)
                nc.sync.dma_start(mlw_sb[:], mlw_aug_in[:])
                muw_sb = wp.tile([2 * HID, HID], F32)
                nc.sync.dma_start(muw_sb[:], muw_in[:])
                mub_sb = wp.tile([HID, 1], F32)
                nc.sync.dma_start(mub_sb[:], mub_in[:])

                mstate = {}

                def mfront(b):
                    xgm_blk = wp.tile([P, K_sage * XG], BF, tag="xgm", bufs=3)
                    gather_block(xgm_blk, x_gather_in, sage_idx, b, sage_ch,
                                 SCOLS, XG)
                    xsT_ps = pp.tile([D_IN + 1, P], F32, tag="xs", bufs=2,
                                     space="PSUM")
                    for k in range(K_sage):
                        nc.tensor.matmul(
                            xsT_ps[:],
                            lhsT=xgm_blk[:, k * XG:k * XG + D_IN + 1],
                            rhs=ssel_sb[:, (b * K_sage + k) * P:
                                        (b * K_sage + k + 1) * P],
                            start=(k == 0), stop=(k == K_sage - 1))
                    xsT_sb = wp.tile([D_IN + 1, P], F32, tag="xsT", bufs=3)
                    nc.vector.tensor_copy(xsT_sb[:], xsT_ps[:])
                    mstate[b] = xsT_sb

                def mback(b):
                    xsT_sb = mstate.pop(b)
                    zT_ps = pp.tile([2 * HID, P], F32, tag="zT", bufs=2,
                                    space="PSUM")
                    nc.tensor.matmul(zT_ps[:HID, :], lhsT=mlw_sb[:],
                                     rhs=xT_sb[:, b * P:(b + 1) * P],
                                     start=True, stop=True)
                    nc.tensor.matmul(zT_ps[HID:, :], lhsT=mlw_sb[:],
                                     rhs=xsT_sb[:], start=True, stop=True)
                    zcatT = wp.tile([2 * HID, P], F32, tag="zcatT", bufs=2)
                    nc.vector.tensor_copy(zcatT[:], zT_ps[:])
                    h1T_ps = pp.tile([HID, P], F32, tag="h1T", bufs=2,
                                     space="PSUM")
                    nc.tensor.matmul(h1T_ps[:], lhsT=muw_sb[:], rhs=zcatT[:],
                                     start=True, stop=True)
                    nc.scalar.activation(hT1[:, b * P:(b + 1) * P], h1T_ps[:],
                                         AF.Relu, bias=mub_sb[:])

                for b in range(BLOCKS + 1):
                    if b < BLOCKS:
                        mfront(b)
                    if b >= 1:
                        mback(b - 1)

            # =========================================================
            # GAT GEMM phase: own xl -> DRAM bounce; xr/res -> SBUF resident
            # layer 1 contracts 64 (1 chunk), layers 2/3 contract 512 (4)
            # =========================================================
            def gemm_own(layer, hT_src, nchunk, xl_bounce, start_ag):
                ind = HID if layer == 1 else HC
                with (
                    tc.tile_pool(name=f"gw{layer}", bufs=1) as wpool,
                    tc.tile_pool(name=f"gp{layer}", bufs=1, space="PSUM") as pp,
                ):
                    wl_sb = wpool.tile([P, nchunk * HC], BF, tag="wl")
                    wr_sb = wpool.tile([P, nchunk * HC], BF, tag="wr")
                    wres_sb = wpool.tile([P, nchunk * HC], BF, tag="wres")
                    for kc in range(nchunk):
                        rows = slice(kc * P, kc * P + min(P, ind - kc * P))
                        nr = rows.stop - rows.start
                        nc.sync.dma_start(wl_sb[:nr, kc * HC:(kc + 1) * HC],
                                          wls[layer][rows, :])
                        nc.sync.dma_start(wr_sb[:nr, kc * HC:(kc + 1) * HC],
                                          wrs[layer][rows, :])
                        nc.sync.dma_start(wres_sb[:nr, kc * HC:(kc + 1) * HC],
                                          wress[layer][rows, :])
                    cd = HID if layer == 1 else P  # contract rows per chunk

                    def lhs_of(b, kc):
                        if layer == 1:
                            return hT_src[:, b * P:(b + 1) * P]
                        return hT_src[:, kc * NPAD + b * P:
                                      kc * NPAD + (b + 1) * P]

                    # pass 1: xl only, so the AllGather can start early
                    for b in range(BLOCKS):
                        xl_ps = pp.tile([P, HC], F32, tag="xl", bufs=2,
                                        space="PSUM")
                        for kc in range(nchunk):
                            nc.tensor.matmul(
                                xl_ps[:], lhsT=lhs_of(b, kc),
                                rhs=wl_sb[:cd, kc * HC:(kc + 1) * HC],
                                start=(kc == 0), stop=(kc == nchunk - 1))
                        xl_sb = wpool.tile([P, HC], BF, tag="xl_sb", bufs=3)
                        nc.scalar.copy(xl_sb[:], xl_ps[:])
                        nc.sync.dma_start(xl_bounce[b * P:(b + 1) * P, :],
                                          xl_sb[:])
                    start_ag()
                    # pass 2: xr/res, overlapped with the collective
                    for b in range(BLOCKS):
                        xr_ps = pp.tile([P, HC], F32, tag="xr", bufs=2,
                                        space="PSUM")
                        res_ps = pp.tile([P, HC], F32, tag="res", bufs=2,
                                         space="PSUM")
                        for kc in range(nchunk):
                            lhs = lhs_of(b, kc)
                            nc.tensor.matmul(
                                xr_ps[:], lhsT=lhs,
                                rhs=wr_sb[:cd, kc * HC:(kc + 1) * HC],
                                start=(kc == 0), stop=(kc == nchunk - 1))
                            nc.tensor.matmul(
                                res_ps[:], lhsT=lhs,
                                rhs=wres_sb[:cd, kc * HC:(kc + 1) * HC],
                                start=(kc == 0), stop=(kc == nchunk - 1))
                        nc.scalar.copy(xr_all[:, b * HC:(b + 1) * HC],
                                       xr_ps[:])
                        nc.scalar.copy(res_all[:, b * HC:(b + 1) * HC],
                                       res_ps[:])

            # =========================================================
            # GAT edge phase — software pipelined: FRONT(b) overlaps BACK(b-1)
            # =========================================================
            def gat_edge_phase(layer, xl_dram, hT_next, h3_bounce=None):
                K = K_gat
                gat_g3 = _groupsn(K, 3)
                with (
                    tc.tile_pool(name=f"edge_sb{layer}", bufs=1) as wp,
                    tc.tile_pool(name=f"edge_ps{layer}", bufs=1,
                                 space="PSUM") as pp,
                ):
                    att_sb = wp.tile([P, HC], BF, tag="att")
                    nc.sync.dma_start(att_sb[:], atts[layer][:])
                    bias_sb = wp.tile([P, HC], BF, tag="bias")
                    nc.sync.dma_start(bias_sb[:], brs[layer][:])
                    state = {}

                    def front(b):
                        xg_blk = wp.tile([P, K * HC], BF, tag="xg_blk", bufs=3)
                        gather_block(xg_blk, xl_dram, gat_idx, b, gat_ch,
                                     GCOLS, HC)
                        selT_blk = wp.tile([P, K * P], BF, tag="selT", bufs=2)
                        nc.sync.dma_start(selT_blk[:],
                                          gat_selT_in[:, b * K * P:
                                                      (b + 1) * K * P])
                        sel_blk = wp.tile([P, K * P], BF, tag="sel", bufs=3)
                        nc.sync.dma_start(sel_blk[:],
                                          gat_sel_in[:, b * K * P:
                                                     (b + 1) * K * P])

                        # z = selT^T @ xr_blk + xg (PSUM); lk = Prelu(z, 0.2)
                        lk = wp.tile([P, K * HC], BF, tag="lk", bufs=2)
                        for (g0, ng) in gat_g3:
                            zg_ps = pp.tile([P, 3 * HC], F32, tag="zg",
                                            bufs=2, space="PSUM")
                            for k in range(g0, g0 + ng):
                                sl = slice((k - g0) * HC, (k - g0 + 1) * HC)
                                nc.tensor.matmul(
                                    zg_ps[:, sl],
                                    lhsT=selT_blk[:, k * P:(k + 1) * P],
                                    rhs=xr_all[:, b * HC:(b + 1) * HC],
                                    start=True, stop=False)
                                nc.tensor.matmul(
                                    zg_ps[:, sl], lhsT=ident_bf[:],
                                    rhs=xg_blk[:, k * HC:(k + 1) * HC],
                                    start=False, stop=True)
                            nc.scalar.activation(
                                lk[:, g0 * HC:(g0 + ng) * HC],
                                zg_ps[:, :ng * HC], AF.Prelu, alpha=0.2)

                        # alpha = sum_c att * lk (per head); am in-place in lk
                        # group-split so the DVE chain trails each Prelu group
                        t1 = wp.tile([P, K * HEADS * 32], BF, tag="t1", bufs=1)
                        t2 = wp.tile([P, K * HEADS * 16], BF, tag="t2", bufs=1)
                        alpha_blk = wp.tile([P, K * HEADS], F32, tag="alpha",
                                            bufs=1)
                        for (g0, ng) in gat_g3:
                            gs = slice(g0 * HC, (g0 + ng) * HC)
                            nc.vector.tensor_mul(
                                lk[:, gs].rearrange("p (k d) -> p k d", k=ng),
                                lk[:, gs].rearrange("p (k d) -> p k d", k=ng),
                                att_sb[:].unsqueeze(1).to_broadcast(
                                    [P, ng, HC]))
                            amv = lk[:, gs].rearrange("p (s c) -> p s c", c=HID)
                            t1g = t1[:, g0 * HEADS * 32:(g0 + ng) * HEADS * 32]
                            nc.vector.tensor_add(
                                t1g.rearrange("p (s c) -> p s c", c=32),
                                amv[:, :, 0:32], amv[:, :, 32:64])
                            t1v = t1g.rearrange("p (s c) -> p s c", c=32)
                            t2g = t2[:, g0 * HEADS * 16:(g0 + ng) * HEADS * 16]
                            nc.vector.tensor_add(
                                t2g.rearrange("p (s c) -> p s c", c=16),
                                t1v[:, :, 0:16], t1v[:, :, 16:32])
                            nc.vector.reduce_sum(
                                out=alpha_blk[:, g0 * HEADS:(g0 + ng) * HEADS],
                                in_=t2g.rearrange("p (k h c) -> p k h c",
                                                  k=ng, c=16),
                                axis=mybir.AxisListType.X)
                        ea_blk = wp.tile([P, K * HEADS], F32, tag="ea", bufs=1)
                        nc.scalar.activation(ea_blk[:], alpha_blk[:], AF.Exp)
                        eam_blk = wp.tile([P, K * HEADS], BF, tag="eam", bufs=3)
                        nc.vector.tensor_mul(
                            eam_blk[:].rearrange("p (k h) -> p k h", k=K),
                            ea_blk[:].rearrange("p (k h) -> p k h", k=K),
                            gmask[:, b * K:(b + 1) * K].unsqueeze(2)
                                .to_broadcast([P, K, HEADS]))
                        # expand eam over HID (scalar), rhs mul trails per group
                        eam_exp = wp.tile([P, K * HC], BF, tag="lk", bufs=2)
                        for (g0, ng) in gat_g3:
                            gs = slice(g0 * HC, (g0 + ng) * HC)
                            nc.scalar.activation(
                                eam_exp[:, gs].rearrange(
                                    "p (s c) -> p s c", c=HID),
                                eam_blk[:, g0 * HEADS:(g0 + ng) * HEADS]
                                    .unsqueeze(2)
                                    .to_broadcast([P, ng * HEADS, HID]),
                                AF.Copy)
                            nc.vector.tensor_mul(xg_blk[:, gs], xg_blk[:, gs],
                                                 eam_exp[:, gs])
                        state[b] = (xg_blk, sel_blk, eam_blk)

                    def back(b):
                        rhs_blk, sel_blk, eam_blk = state.pop(b)
                        out_ps = pp.tile([P, HC], F32, tag="outps", bufs=1,
                                         space="PSUM")
                        den_ps = pp.tile([P, HEADS], F32, tag="denps", bufs=1,
                                         space="PSUM")
                        for k in range(K):
                            nc.tensor.matmul(out_ps[:],
                                             lhsT=sel_blk[:, k * P:(k + 1) * P],
                                             rhs=rhs_blk[:, k * HC:(k + 1) * HC],
                                             start=(k == 0), stop=(k == K - 1))
                            nc.tensor.matmul(den_ps[:],
                                             lhsT=sel_blk[:, k * P:(k + 1) * P],
                                             rhs=eam_blk[:, k * HEADS:
                                                         (k + 1) * HEADS],
                                             start=(k == 0), stop=(k == K - 1))

                        den_sb = wp.tile([P, HEADS], F32, tag="den", bufs=2)
                        nc.vector.tensor_scalar_add(den_sb[:], den_ps[:], 1e-16)
                        rec = wp.tile([P, HEADS], F32, tag="rec", bufs=2)
                        nc.vector.reciprocal(rec[:], den_sb[:])
                        # o in bf16: out/den is pre-activation, bf16 is enough
                        o = wp.tile([P, HC], BF, tag="o", bufs=2)
                        nc.vector.tensor_mul(
                            o[:].rearrange("p (h c) -> p h c", c=HID),
                            out_ps[:].rearrange("p (h c) -> p h c", c=HID),
                            rec[:].unsqueeze(2).to_broadcast([P, HEADS, HID]))
                        nc.vector.tensor_add(o[:], o[:],
                                             res_all[:, b * HC:(b + 1) * HC])
                        nc.vector.tensor_add(o[:], o[:], bias_sb[:])
                        hn = wp.tile([P, HC], BF, tag="hn", bufs=2)
                        if layer == 2:
                            # Prelu(0.01) == Lrelu but shares the Prelu act
                            # table already loaded for the zg groups (saves
                            # an ACT_TABLE_LOAD per block)
                            nc.scalar.activation(hn[:], o[:], AF.Prelu,
                                                 alpha=0.01)
                        else:
                            neg = wp.tile([P, HC], BF, tag="neg", bufs=2)
                            nc.vector.tensor_scalar_min(neg[:], o[:], 0.0)
                            nc.scalar.activation(neg[:], neg[:], AF.Exp)
                            nc.vector.tensor_scalar_max(hn[:], o[:], 0.0)
                            nc.vector.tensor_add(hn[:], hn[:], neg[:])
                            nc.vector.tensor_scalar_add(hn[:], hn[:], -1.0)
                        if h3_bounce is not None:
                            nc.sync.dma_start(h3_bounce[b * P:(b + 1) * P, :],
                                              hn[:])
                        # hT_next via DMA-transpose (off the tensor engine)
                        for ch in range(4):
                            nc.sync.dma_start(
                                hT_next[:, ch * NPAD + b * P:
                                        ch * NPAD + (b + 1) * P],
                                hn[:, ch * P:(ch + 1) * P], transpose=True)

                    for b in range(BLOCKS + 2):
                        if b < BLOCKS:
                            front(b)
                        if b >= 2:
                            back(b - 2)

            def make_ag(src, dst):
                def start_ag():
                    nc.gpsimd.collective_compute(
                        "AllGather", mybir.AluOpType.bypass,
                        replica_groups=[list(range(NCORES))],
                        ins=[src.opt()], outs=[dst.opt()])
                return start_ag

            # ===================== GAT1 =====================
            xl1_b = dr.tile([NPAD, HC], BF)
            xl1_full = dr.tile([NFULL, HC], BF, addr_space="Shared")
            gemm_own(1, hT1, 1, xl1_b, make_ag(xl1_b, xl1_full))
            gat_edge_phase(1, xl1_full, hT_a)

            # ===================== GAT2 =====================
            xl2_b = dr.tile([NPAD, HC], BF)
            xl2_full = dr.tile([NFULL, HC], BF, addr_space="Shared")
            gemm_own(2, hT_a, 4, xl2_b, make_ag(xl2_b, xl2_full))
            gat_edge_phase(2, xl2_full, hT_b)

            # ===================== GAT3 =====================
            xl3_b = dr.tile([NPAD, HC], BF)
            xl3_full = dr.tile([NFULL, HC], BF, addr_space="Shared")
            h3_bounce = dr.tile([NPAD, HC], BF)
            h3_full = dr.tile([NFULL, HC], BF, addr_space="Shared")
            gemm_own(3, hT_b, 4, xl3_b, make_ag(xl3_b, xl3_full))
            gat_edge_phase(3, xl3_full, hT_a, h3_bounce=h3_bounce)

            nc.gpsimd.collective_compute(
                "AllGather", mybir.AluOpType.bypass,
                replica_groups=[list(range(NCORES))],
                ins=[h3_bounce.opt()], outs=[h3_full.opt()])

            # =========================================================
            # SAGE + pooling
            # =========================================================
            pool_b = dr.tile([G, G], F32)
            pool_full = dr.tile([G, G], F32, addr_space="Shared")
            with tc.tile_pool(name="sg_sb", bufs=1) as wp:
                ssel_sb2 = wp.tile([P, TS * P], BF)
                nc.sync.dma_start(ssel_sb2[:], sage_sel_in[:])
                swn_sb = wp.tile([P, 4 * HID], BF)
                swr_sb = wp.tile([P, 4 * HID], BF)
                for kc in range(4):
                    nc.sync.dma_start(swn_sb[:, kc * HID:(kc + 1) * HID],
                                      swn_in[kc * P:(kc + 1) * P, :])
                    nc.sync.dma_start(swr_sb[:, kc * HID:(kc + 1) * HID],
                                      swr_in[kc * P:(kc + 1) * P, :])
                sbn_sb = wp.tile([P, HID], F32)
                nc.sync.dma_start(sbn_sb[:], sbn_in[:])

                with (
                    tc.tile_pool(name="sg_ps", bufs=1, space="PSUM") as pp,
                    tc.tile_pool(name="pool_ps_pool", bufs=1, space="PSUM") as plp,
                ):
                    pool_ps = plp.tile([G, G], F32, space="PSUM")
                    for b in range(BLOCKS):
                        hg_blk = wp.tile([P, K_sage * HC], BF, tag="hg", bufs=2)
                        gather_block(hg_blk, h3_full, sage_idx, b, sage_ch,
                                     SCOLS, HC)
                        agg_ps = pp.tile([P, HC], F32, tag="agg", bufs=2,
                                         space="PSUM")
                        for k in range(K_sage):
                            nc.tensor.matmul(
                                agg_ps[:],
                                lhsT=ssel_sb2[:, (b * K_sage + k) * P:
                                              (b * K_sage + k + 1) * P],
                                rhs=hg_blk[:, k * HC:(k + 1) * HC],
                                start=(k == 0), stop=(k == K_sage - 1))
                        mean = wp.tile([P, HC], BF, tag="mean", bufs=2)
                        nc.vector.tensor_mul(
                            mean[:], agg_ps[:],
                            rcnt[:, b:b + 1].to_broadcast([P, HC]))
                        mT_ps = pp.tile([P, 4 * P], BF, tag="mT", bufs=2,
                                        space="PSUM")
                        for kc in range(4):
                            nc.tensor.transpose(
                                mT_ps[:, kc * P:(kc + 1) * P],
                                mean[:, kc * P:(kc + 1) * P], ident_bf[:])
                        mT_sb = wp.tile([P, 4 * P], BF, tag="mT_sb", bufs=2)
                        nc.vector.tensor_copy(mT_sb[:], mT_ps[:])
                        sage_ps = pp.tile([P, HID], F32, tag="sage", bufs=2,
                                          space="PSUM")
                        for kc in range(4):
                            nc.tensor.matmul(
                                sage_ps[:], lhsT=mT_sb[:, kc * P:(kc + 1) * P],
                                rhs=swn_sb[:, kc * HID:(kc + 1) * HID],
                                start=(kc == 0), stop=False)
                            lhs_h = hT_a[:, kc * NPAD + b * P:
                                         kc * NPAD + (b + 1) * P]
                            nc.tensor.matmul(
                                sage_ps[:], lhsT=lhs_h,
                                rhs=swr_sb[:, kc * HID:(kc + 1) * HID],
                                start=False, stop=(kc == 3))
                        sage_sb = wp.tile([P, HID], BF, tag="sage_sb", bufs=2)
                        nc.vector.tensor_add(sage_sb[:], sage_ps[:], sbn_sb[:])
                        nc.scalar.activation(sage_sb[:], sage_sb[:], AF.Relu)
                        nc.tensor.matmul(pool_ps[:],
                                         lhsT=B_sb[:, b * G:(b + 1) * G],
                                         rhs=sage_sb[:], start=(b == 0),
                                         stop=(b == BLOCKS - 1))

                    pool_sb = wp.tile([G, G], F32)
                    nc.vector.tensor_copy(pool_sb[:], pool_ps[:])
                    nc.sync.dma_start(pool_b[:], pool_sb[:])

                nc.gpsimd.collective_compute(
                    "AllReduce", mybir.AluOpType.add,
                    replica_groups=[list(range(NCORES))],
                    ins=[pool_b.opt()], outs=[pool_full.opt()])

                with tc.tile_pool(name="head_ps", bufs=1, space="PSUM") as pp:
                    poolf = wp.tile([G, G], F32)
                    nc.sync.dma_start(poolf[:], pool_full[:])
                    nc.vector.tensor_mul(poolf[:], poolf[:],
                                         rgc[:].to_broadcast([G, G]))
                    pT_ps = pp.tile([G, G], F32, tag="pT", space="PSUM")
                    nc.tensor.transpose(pT_ps[:], poolf[:], ident[:G, :G])
                    pT_sb = wp.tile([G, G], F32)
                    nc.vector.tensor_copy(pT_sb[:], pT_ps[:])
                    ow_sb = wp.tile([HID, 2], F32)
                    nc.sync.dma_start(ow_sb[:], ow_in[:])
                    ob_sb = wp.tile([G, 2], F32)
                    nc.sync.dma_start(ob_sb[:], ob_in[:])
                    lg_ps = pp.tile([G, 2], F32, tag="lg", space="PSUM")
                    nc.tensor.matmul(lg_ps[:], lhsT=pT_sb[:], rhs=ow_sb[:],
                                     start=True, stop=True)
                    lg = wp.tile([G, 2], F32)
                    nc.vector.tensor_add(lg[:], lg_ps[:], ob_sb[:])
                    mx = wp.tile([G, 1], F32)
                    nc.vector.reduce_max(out=mx[:], in_=lg[:],
                                         axis=mybir.AxisListType.X)
                    zm = wp.tile([G, 2], F32)
                    nc.vector.tensor_sub(zm[:], lg[:], mx[:].to_broadcast([G, 2]))
                    ez = wp.tile([G, 2], F32)
                    nc.scalar.activation(ez[:], zm[:], AF.Exp)
                    s = wp.tile([G, 1], F32)
                    nc.vector.reduce_sum(out=s[:], in_=ez[:],
                                         axis=mybir.AxisListType.X)
                    ls = wp.tile([G, 1], F32)
                    nc.scalar.activation(ls[:], s[:], AF.Ln)
                    res_out = wp.tile([G, 2], F32)
                    nc.vector.tensor_sub(res_out[:], zm[:],
                                         ls[:].to_broadcast([G, 2]))
                    nc.sync.dma_start(out[:], res_out[:])

    nc.compile()
    return nc


def _make_in_maps(pre):
    w = pre["weights"]
    in_maps = []
    for c in range(NCORES):
        pc = pre["per_core"][c]
        m = {
            "x_gather": pre["x_gather"],
            "xT_aug": pre["xT_aug"][c],
            "gat_idx16": pc["gat_idx16"],
            "sage_idx16": pc["sage_idx16"],
            "gat_selT": pc["gat_selT"], "gat_sel": pc["gat_sel"],
            "gat_mask": pc["gat_mask"],
            "sage_sel": pc["sage_sel"], "recip_cnt": pc["recip_cnt"],
            "B_onehot": pre["B_all"][c],
            "recip_gcnt": pre["recip_gcnt"],
            "mlw_aug": w["mlw_aug"], "muw": w["muw"], "mub_col": w["mub_col"],
            "sage_wn": w["sage_wn"], "sage_wr": w["sage_wr"],
            "sbn_rep": w["sbn_rep"],
            "out_w": w["out_w"], "ob_rep": w["ob_rep"],
        }
        for i in (1, 2, 3):
            m[f"wl{i}"] = w[f"wl{i}"]
            m[f"wr{i}"] = w[f"wr{i}"]
            m[f"wres{i}"] = w[f"wres{i}"]
            m[f"att_rep{i}"] = w[f"att_rep{i}"]
            m[f"b_rep{i}"] = w[f"b_rep{i}"]
        in_maps.append(m)
    return in_maps


def kernel(**inputs):
    pre = _preprocess(inputs)
    key = (pre["K_gat"], pre["K_sage"])
    if key not in _CACHE:
        _CACHE[key] = _build(*key)
    nc = _CACHE[key]
    in_maps = _make_in_maps(pre)
    res = bass_utils.run_bass_kernel_spmd(nc, in_maps, core_ids=list(range(NCORES)))
    return res.results[0]["out"]
